# revision 14
# baseline (speedup 1.0000x reference)
"""BiLSTM (2-layer, masked/ragged) Trainium2 kernel.

Sharding: 8 cores = 2 directions x 4 batch shards (16 each). Backward
direction cores receive time-reversed inputs from the host, so the device
program is direction-agnostic SPMD. Layer-0 direction outputs are swapped
between fwd/bwd partner cores with an 8-core AllGather of time-reversed
copies; each core then computes layer-1 input projections from its own +
partner halves and runs the layer-1 scan.

All matmuls in bf16 (weights stationary, gates on PSUM partitions, batch on
the free dim), cell state and elementwise chain in fp32.
"""

import os
import numpy as np
import ml_dtypes

import concourse.bass as bass
import concourse.bacc as bacc
import concourse.mybir as mybir
import concourse.tile as tile
from concourse import bass_utils

bf16 = ml_dtypes.bfloat16
f32 = mybir.dt.float32
bf = mybir.dt.bfloat16

T, B, D, H = 512, 64, 512, 512
NCORES = 8
BS = B // 4  # 16, batch shard per core
G = 4 * H  # 2048 gates
GT = G // 128  # 16 gate tiles
KH = H // 128  # 4 k-chunks for hidden contraction
TC = 32  # timesteps per phase chunk
NC_CHUNKS = T // TC  # 16

_compiled = {}


def _build(t_steps=T, dbg=False):
    nc = bacc.Bacc("TRN2", target_bir_lowering=False, debug=False,
                   num_devices=NCORES)
    nchunks = t_steps // TC

    # ---- per-core external inputs ----
    xT = nc.dram_tensor("xT", (D, t_steps, BS), bf, kind="ExternalInput")
    maskd = nc.dram_tensor("maskd", (t_steps, KH * BS), bf, kind="ExternalInput")
    whh0T = nc.dram_tensor("whh0T", (KH, 128, G), bf, kind="ExternalInput")
    wih0T = nc.dram_tensor("wih0T", (KH, 128, G), bf, kind="ExternalInput")
    whh1T = nc.dram_tensor("whh1T", (KH, 128, G), bf, kind="ExternalInput")
    wih1oT = nc.dram_tensor("wih1oT", (KH, 128, G), bf, kind="ExternalInput")
    wih1pT = nc.dram_tensor("wih1pT", (KH, 128, G), bf, kind="ExternalInput")
    b0c = nc.dram_tensor("b0c", (GT, 128), f32, kind="ExternalInput")
    b1c = nc.dram_tensor("b1c", (GT, 128), f32, kind="ExternalInput")
    y1 = nc.dram_tensor("y1", (t_steps, 128, KH * BS), f32, kind="ExternalOutput")
    if dbg:
        xwb0o = nc.dram_tensor("xwb0o", (t_steps, 128, GT * BS), bf, kind="ExternalOutput")
        xwb1o = nc.dram_tensor("xwb1o", (t_steps, 128, GT * BS), bf, kind="ExternalOutput")
        y0o = nc.dram_tensor("y0o", (t_steps, 128, KH * BS), bf, kind="ExternalOutput")
        ago = nc.dram_tensor("ago", (NCORES * t_steps, 128, KH * BS), bf, kind="ExternalOutput")

    with tile.TileContext(nc) as tc:
        with (
            tc.tile_pool(name="wpool", bufs=1) as wpool,
            tc.tile_pool(name="xpool", bufs=3) as xpool,
            tc.tile_pool(name="gpool", bufs=3) as gpool,
            tc.tile_pool(name="spool", bufs=5) as spool,
            tc.tile_pool(name="state", bufs=1) as state,
            tc.tile_pool(name="psA", bufs=2, space="PSUM") as psA,
            tc.tile_pool(name="psS", bufs=3, space="PSUM") as psS,
            tc.tile_pool(name="dram", bufs=1, space="DRAM") as dram,
        ):
            # ---- internal DRAM ----
            xwb0 = dram.tile([t_steps, 128, GT * BS], bf)
            xwb1 = dram.tile([t_steps, 128, GT * BS], bf)
            y0 = dram.tile([t_steps, 128, KH * BS], bf)
            y0x = dram.tile([t_steps, 128, KH * BS], bf)
            ag = dram.tile([NCORES * t_steps, 128, KH * BS], bf)

            # ---- resident weights ----
            def load_w(name, src):
                t = wpool.tile([128, KH * G], bf, tag=name)
                for k in range(KH):
                    nc.sync.dma_start(t[:, k * G:(k + 1) * G], src.ap()[k])
                return t

            whh0_sb = load_w("whh0", whh0T)
            wih0_sb = load_w("wih0", wih0T)
            whh1_sb = load_w("whh1", whh1T)
            wih1o_sb = load_w("wih1o", wih1oT)
            wih1p_sb = load_w("wih1p", wih1pT)
            bias_sb = wpool.tile([128, 2 * GT], f32, tag="bias")
            nc.sync.dma_start(bias_sb[:, 0:GT], b0c.ap().transpose([1, 0]))
            nc.sync.dma_start(bias_sb[:, GT:2 * GT], b1c.ap().transpose([1, 0]))

            # ---- phase A / D: input projections -> xwb dram ----
            def proj(dst, w_sbs, srcs, bias_col, dbg_dst=None):
                """dst[t,p,g*BS+b] = sum_j srcs[j] @ w_sbs-chunks + bias."""
                nk = len(w_sbs) * KH
                for ncnk in range(nchunks):
                    t0 = ncnk * TC
                    rhs = xpool.tile([128, nk, TC, BS], bf, tag="projx")
                    ji = 0
                    for w_sb, src in zip(w_sbs, srcs):
                        for k in range(KH):
                            nc.sync.dma_start(rhs[:, ji], src(k, t0))
                            ji += 1
                    for g in range(GT):
                        ps = psA.tile([128, TC * BS], f32, tag="psA")
                        ji = 0
                        for w_sb in w_sbs:
                            for k in range(KH):
                                nc.tensor.matmul(
                                    ps[:],
                                    w_sb[:, k * G + g * 128: k * G + (g + 1) * 128],
                                    rhs[:, ji],
                                    start=(ji == 0),
                                    stop=(ji == nk - 1),
                                )
                                ji += 1
                        g_sb = gpool.tile([128, TC * BS], bf, tag="projg")
                        nc.scalar.activation(
                            g_sb[:], ps[:], mybir.ActivationFunctionType.Identity,
                            bias=bias_sb[:, bias_col + g: bias_col + g + 1],
                        )
                        nc.sync.dma_start(
                            dst[t0:t0 + TC, :, g * BS:(g + 1) * BS].transpose([1, 0, 2]),
                            g_sb[:].rearrange("p (t b) -> p t b", t=TC),
                        )
                        if dbg_dst is not None:
                            nc.sync.dma_start(
                                dbg_dst.ap()[t0:t0 + TC, :, g * BS:(g + 1) * BS].transpose([1, 0, 2]),
                                g_sb[:].rearrange("p (t b) -> p t b", t=TC),
                            )

            proj(
                xwb0, [wih0_sb],
                [lambda k, t0: xT.ap()[k * 128:(k + 1) * 128, t0:t0 + TC, :]],
                bias_col=0,
                dbg_dst=xwb0o if dbg else None,
            )

            # ---- scan helper ----
            def scan(xwb, whh_sb, y_dst, yx_dst, out_f32, dbg_dst=None):
                h2f = state.tile([128, KH * BS], f32, tag="h2f")
                cst = state.tile([128, KH * BS], f32, tag="cst")
                nc.gpsimd.memset(h2f[:], 0.0)
                nc.gpsimd.memset(cst[:], 0.0)
                for t in range(t_steps):
                    m_sb = spool.tile([128, KH * BS], bf, tag="m")
                    nc.sync.dma_start(
                        m_sb[:], maskd.ap()[t:t + 1, :].broadcast_to([128, KH * BS]))
                    xw_sb = spool.tile([128, GT * BS], bf, tag="xw")
                    nc.sync.dma_start(xw_sb[:], xwb[t])
                    h_in = spool.tile([128, KH * BS], bf, tag="hin")
                    nc.vector.tensor_mul(h_in[:], h2f[:], m_sb[:])
                    nc.vector.tensor_mul(cst[:], cst[:], m_sb[:])
                    ps = psS.tile([128, GT * BS], f32, tag="psS")
                    for g in range(GT):
                        for k in range(KH):
                            nc.tensor.matmul(
                                ps[:, g * BS:(g + 1) * BS],
                                whh_sb[:, k * G + g * 128: k * G + (g + 1) * 128],
                                h_in[:, k * BS:(k + 1) * BS],
                                start=(k == 0),
                                stop=(k == KH - 1),
                            )
                    nc.vector.tensor_add(ps[:], ps[:], xw_sb[:])
                    # gates: cols [0:2H') i,f | [2H':3H') g | [3H':4H') o  (H'=KH*BS)
                    HB = KH * BS
                    sif = spool.tile([128, 2 * HB], f32, tag="sif")
                    nc.scalar.activation(
                        sif[:], ps[:, 0:2 * HB], mybir.ActivationFunctionType.Sigmoid)
                    tng = spool.tile([128, HB], f32, tag="tng")
                    nc.scalar.activation(
                        tng[:], ps[:, 2 * HB:3 * HB], mybir.ActivationFunctionType.Tanh)
                    so = spool.tile([128, HB], f32, tag="so")
                    nc.scalar.activation(
                        so[:], ps[:, 3 * HB:4 * HB], mybir.ActivationFunctionType.Sigmoid)
                    ig = spool.tile([128, HB], f32, tag="ig")
                    nc.vector.tensor_mul(ig[:], sif[:, 0:HB], tng[:])
                    fc = spool.tile([128, HB], f32, tag="fc")
                    nc.vector.tensor_mul(fc[:], sif[:, HB:2 * HB], cst[:])
                    nc.vector.tensor_add(cst[:], fc[:], ig[:])
                    tc2 = spool.tile([128, HB], f32, tag="tc2")
                    nc.scalar.activation(
                        tc2[:], cst[:], mybir.ActivationFunctionType.Tanh)
                    nc.vector.tensor_mul(h2f[:], so[:], tc2[:])
                    y_sb = spool.tile([128, HB], f32 if out_f32 else bf, tag="y")
                    nc.vector.tensor_mul(y_sb[:], h2f[:], m_sb[:])
                    nc.sync.dma_start(y_dst[t], y_sb[:])
                    if yx_dst is not None:
                        nc.sync.dma_start(yx_dst[t_steps - 1 - t], y_sb[:])
                    if dbg_dst is not None:
                        nc.sync.dma_start(dbg_dst.ap()[t], y_sb[:])

            scan(xwb0, whh0_sb, y0, y0x, out_f32=False,
                 dbg_dst=y0o if dbg else None)

            # ---- exchange ----
            nc.gpsimd.collective_compute(
                "AllGather", mybir.AluOpType.bypass,
                ins=[y0x.opt()], outs=[ag.opt()],
                replica_groups=[list(range(NCORES))],
            )
            partner_row = nc.snap(((nc.partition_id() + 4) % 8) * t_steps)

            if dbg:
                nc.sync.dma_start(ago.ap()[:], ag[:])

            proj(
                xwb1, [wih1o_sb, wih1p_sb],
                [
                    lambda k, t0: y0[t0:t0 + TC, :, k * BS:(k + 1) * BS].transpose([1, 0, 2]),
                    lambda k, t0: ag[bass.ds(partner_row + t0, TC), :, k * BS:(k + 1) * BS].transpose([1, 0, 2]),
                ],
                bias_col=GT,
                dbg_dst=xwb1o if dbg else None,
            )

            scan(xwb1, whh1_sb, y1.ap(), None, out_f32=True)

    nc.compile()
    return nc


def _prep_inputs(x, lengths, weights, t_steps=T):
    """Build the 8 per-core input maps."""
    active = (np.arange(T)[:, None] < np.asarray(lengths)[None, :]).astype(np.float32)
    in_maps = []
    for c in range(NCORES):
        d, s = c // 4, c % 4
        bsl = slice(s * BS, (s + 1) * BS)
        pre = "f" if d == 0 else "b"
        xs = np.asarray(x[:, bsl, :], np.float32)
        am = active[:, bsl]
        if d == 1:
            xs = xs[::-1]
            am = am[::-1]
        xs = xs[:t_steps]
        am = am[:t_steps]

        W_ih0 = np.asarray(weights[f"{pre}W_ih0"], np.float32)
        W_hh0 = np.asarray(weights[f"{pre}W_hh0"], np.float32)
        W_ih1 = np.asarray(weights[f"{pre}W_ih1"], np.float32)
        W_hh1 = np.asarray(weights[f"{pre}W_hh1"], np.float32)
        b0 = np.asarray(weights[f"{pre}b0"], np.float32)
        b1 = np.asarray(weights[f"{pre}b1"], np.float32)
        own = W_ih1[:, :512] if d == 0 else W_ih1[:, 512:]
        par = W_ih1[:, 512:] if d == 0 else W_ih1[:, :512]

        in_maps.append({
            "xT": np.ascontiguousarray(xs.transpose(2, 0, 1)).astype(bf16),
            "maskd": np.ascontiguousarray(np.tile(am, (1, KH))).astype(bf16),
            "whh0T": np.ascontiguousarray(W_hh0.T.reshape(KH, 128, G)).astype(bf16),
            "wih0T": np.ascontiguousarray(W_ih0.T.reshape(KH, 128, G)).astype(bf16),
            "whh1T": np.ascontiguousarray(W_hh1.T.reshape(KH, 128, G)).astype(bf16),
            "wih1oT": np.ascontiguousarray(own.T.reshape(KH, 128, G)).astype(bf16),
            "wih1pT": np.ascontiguousarray(par.T.reshape(KH, 128, G)).astype(bf16),
            "b0c": np.ascontiguousarray(b0.reshape(GT, 128)).astype(np.float32),
            "b1c": np.ascontiguousarray(b1.reshape(GT, 128)).astype(np.float32),
        })
    return in_maps


def _assemble(results, t_steps=T):
    out = np.zeros((t_steps, B, 2 * H), np.float32)
    for c in range(NCORES):
        d, s = c // 4, c % 4
        arr = results[c]["y1"].reshape(t_steps, 128, KH, BS)
        if d == 1:
            arr = arr[::-1]
        # [t, p, j, b] -> [t, b, j*128+p]
        blk = arr.transpose(0, 3, 2, 1).reshape(t_steps, BS, H)
        out[:, s * BS:(s + 1) * BS, d * H:(d + 1) * H] = blk
    return out


def kernel(x, lengths, fW_ih0, fW_hh0, fb0, bW_ih0, bW_hh0, bb0,
           fW_ih1, fW_hh1, fb1, bW_ih1, bW_hh1, bb1, _t_steps=T,
           _want_trace=False, _dbg=False):
    weights = dict(fW_ih0=fW_ih0, fW_hh0=fW_hh0, fb0=fb0,
                   bW_ih0=bW_ih0, bW_hh0=bW_hh0, bb0=bb0,
                   fW_ih1=fW_ih1, fW_hh1=fW_hh1, fb1=fb1,
                   bW_ih1=bW_ih1, bW_hh1=bW_hh1, bb1=bb1)
    key = (_t_steps, _dbg)
    if key not in _compiled:
        _compiled[key] = _build(_t_steps, dbg=_dbg)
    nc = _compiled[key]
    in_maps = _prep_inputs(x, lengths, weights, _t_steps)
    res = bass_utils.run_bass_kernel_spmd(
        nc, in_maps, core_ids=list(range(NCORES)), trace=_want_trace)
    out = _assemble(res.results, _t_steps)
    if _want_trace or _dbg:
        kernel.last_results = res
    return out
